# revision 37
# baseline (speedup 1.0000x reference)
"""Trainium2 Bass kernel for nn_Block_CD (dual-stream patch-embed + attention).

v13 design (per 16-sample slice, one stream; tokens t=(s,l), l=25):
  ic [52, 400]: im2col(3x3 conv, 27 rows) + one-hot position rows (25).
  var+eps = ic^T Mv ic (quadratic form; Mv folds mean, eps, 1/256)
    y = Mv @ ic (PE); z = ic*y (DVE); var = ones52 @ z (PE); evac (ACT)
  rs = rsqrt(var)/16 via bf16 fast-inverse-sqrt + 1 Newton step (DVE/Pool)
  ic_s = ic * rs  -> q,k,v matmuls contract 52 and come out LN-normalized
  q,k feature-major [128,(g,400)] from A_q,A_k (PE + plain evacs)
  kbd block-diag k built by strided bf16 copies (DVE 4x / Pool)
  scores: per-sample 128-contraction matmuls; E = exp(SCALE*sc) (ACT)
  den = ones_den @ E (PE); rden = 1/den (DVE); v token-major via ic32_s
  (32-padded) matmuls -> V4 [ (s4,m32), (j,f256) ]; av: per-(s,h) tiny
  matmuls lhsT=V4 slice, rhs=E slice; avn = av*rden (DVE)
  pp = proj^T avn + W_res^T ic (residual+proj_b folded); output DMA'd
  directly from PSUM as f32 (no o2 evac), host converts.
Sharding: pure data parallel, B=8192 over 8 cores.
"""
import sys
sys.path.insert(0, "/opt/trn_rl_repo")
import numpy as np
import ml_dtypes

import concourse.bass as bass
import concourse.mybir as mybir
import concourse.tile as tile
from concourse import bacc, bass_utils
from concourse.bass import ds

bf16 = mybir.dt.bfloat16
f32 = mybir.dt.float32
u16 = mybir.dt.uint16
AF = mybir.ActivationFunctionType
ALU = mybir.AluOpType

DIM = 256
HEADS = 8
HD = 32
L = 25
SCALE = HD ** -0.5
LN_EPS = 1e-5
NCORES = 8
B = 8192
B_LOC = B // NCORES

S_I = 16          # samples per inner slice
N_I = S_I * L     # 400 tokens
U = 32            # slices per hw-loop iteration
STAGGER = 2
KQ_SHARE = 2      # kbd/qv staging rings shared between slices u, u+KQ_SHARE
TOK_B = U * N_I   # 3200 tokens per For_i step

_CACHE = {}


def _to_bf16(a):
    return np.asarray(a, np.float32).astype(ml_dtypes.bfloat16)


def _host_prep(inputs):
    pos = np.asarray(inputs["pos_embed"], np.float64).reshape(L, DIM)
    ln_g = np.asarray(inputs["ln_g"], np.float64)
    ln_b = np.asarray(inputs["ln_b"], np.float64)

    def im2col_ext(img):
        p = np.pad(np.asarray(img, np.float32), ((0, 0), (0, 0), (1, 1), (1, 1)))
        Bn = img.shape[0]
        cols = np.empty((Bn, L, 52), np.float32)
        idx = 0
        for c in range(3):
            for di in range(3):
                for dj in range(3):
                    cols[:, :, idx] = p[:, c, di:di + 5, dj:dj + 5].reshape(Bn, L)
                    idx += 1
        cols[:, :, 27:] = np.eye(L, dtype=np.float32)[None]
        return cols  # [B, 25, 52]

    prep = {}
    for nm, ik, cw, cb, qw, pw, pb in (
        ("x", "x", "conv1_w", "conv1_b", "qkv_x_w", "proj_x_w", "proj_x_b"),
        ("y", "y", "conv2_w", "conv2_b", "qkv_y_w", "proj_y_w", "proj_y_b"),
    ):
        conv_w = np.asarray(inputs[cw], np.float64)
        conv_b = np.asarray(inputs[cb], np.float64)
        qkv_w = np.asarray(inputs[qw], np.float64)
        proj_w = np.asarray(inputs[pw], np.float64)
        proj_b = np.asarray(inputs[pb], np.float64)

        w_emb = np.empty((52, DIM), np.float64)
        w_emb[:27] = conv_w.reshape(DIM, 27).T
        w_emb[27:] = pos + conv_b[None, :]

        # quadratic-form variance matrix: var+eps = c^T Mv c
        m = w_emb.mean(axis=1)                     # [52] row means
        Mv = w_emb @ w_emb.T / DIM - np.outer(m, m)
        Mv[27:, 27:] += LN_EPS * np.eye(L)
        prep[f"mv_{nm}"] = Mv

        c = qkv_w @ ln_b
        assert np.abs(c).max() < 1e-6, "nonzero ln_b fold not supported"
        w_emb_c = w_emb - m[:, None]               # LN mean fold
        wqkv = w_emb_c @ (16.0 * qkv_w * ln_g[None, :]).T   # [52, 768]
        prep[f"aq_{nm}"] = wqkv[:, 0:256]
        prep[f"ak_{nm}"] = wqkv[:, 256:512]
        prep[f"av_{nm}"] = wqkv[:, 512:768]

        wp = proj_w.T                               # [256, 256] lhsT
        prep[f"wproj_{nm}"] = np.concatenate([wp[0:128], wp[128:256]], axis=1)

        w_res = np.empty((52, DIM), np.float64)     # residual + proj_b fold
        w_res[:27] = conv_w.reshape(DIM, 27).T
        w_res[27:] = (conv_b + proj_b)[None, :]
        prep[f"wres_{nm}"] = w_res

        prep[f"ic_{nm}"] = im2col_ext(inputs[ik])

    O = np.zeros((128, 128), np.float32)
    for h in range(4):
        O[h * HD:h * HD + L, h * HD:(h + 1) * HD] = 1.0
    prep["ones_den"] = O
    return prep


def _slice_phases(nc, sb, ps, W, nm, ic, u, kbd, qv):
    """Phase-emitter closures for one 16-sample slice of one stream."""
    st = {}

    def ph_yz():
        # var+eps = ic^T Mv ic  (per token, replicated to 52 partitions)
        yq = ps.tile([52, N_I], f32, tag="st", bufs=1)
        nc.tensor.matmul(yq[:, :], W[f"mv_{nm}"][:, :], ic[:, :], start=True, stop=True)
        z = sb.tile([52, N_I], bf16, tag="z", bufs=4, name=f"z{u}")
        nc.vector.tensor_mul(z[:, :], ic[:, :], yq[:, :])
        st["z"] = z

    def ph_var():
        z = st["z"]
        vq = ps.tile([52, N_I], f32, tag="st", bufs=1)
        nc.tensor.matmul(vq[:, :], W["ones52"][:, :], z[:, :], start=True, stop=True)
        vb = sb.tile([52, N_I], bf16, tag="vb", bufs=4, name=f"vb{u}")
        nc.scalar.activation(vb[:, :], vq[:, :], AF.Copy)
        st["vb"] = vb

    def ph_rsqrt():
        vb = st["vb"]
        # bf16 fast inverse sqrt + 1 Newton step; rs = rsqrt(var+eps)/16
        y0 = sb.tile([52, N_I], bf16, tag="y0", bufs=4, name=f"y0{u}")
        t1 = sb.tile([52, N_I], bf16, tag="t1", bufs=4, name=f"t1{u}")
        rs = sb.tile([52, N_I], bf16, tag="rs", bufs=4, name=f"rs{u}")
        LSR = ALU.logical_shift_right
        nc.vector.tensor_scalar(y0[:, :].bitcast(u16), vb[:, :].bitcast(u16),
                                1, None, LSR)
        nc.gpsimd.tensor_sub(y0[:, :].bitcast(u16), W["magic"][:, :],
                             y0[:, :].bitcast(u16))
        nc.gpsimd.tensor_mul(t1[:, :], vb[:, :], y0[:, :])
        nc.gpsimd.tensor_mul(t1[:, :], t1[:, :], y0[:, :])
        nc.vector.tensor_scalar(t1[:, :], t1[:, :], -0.03125, 0.09375,
                                ALU.mult, ALU.add)
        nc.gpsimd.tensor_mul(rs[:, :], y0[:, :], t1[:, :])
        st["rs"] = rs

    def ph_scale():
        rs = st["rs"]
        ics = sb.tile([52, N_I], bf16, tag="ics", bufs=4, name=f"ics{u}")
        nc.gpsimd.tensor_mul(ics[:, :], ic[:, :], rs[:, :])
        st["ics"] = ics

    def ph_qkv():
        ics = st["ics"]
        q = sb.tile([128, 2 * N_I], bf16, tag="q", bufs=4, name=f"q{u}")
        k = sb.tile([128, 2 * N_I], bf16, tag="k", bufs=4, name=f"k{u}")
        for dst, wk in ((q, f"aq_{nm}"), (k, f"ak_{nm}")):
            p8 = ps.tile([128, 1024], f32, tag="mm8", bufs=1)
            for g in range(2):
                nc.tensor.matmul(p8[:, 512 * g:512 * g + N_I],
                                 W[wk][:, 128 * g:128 * (g + 1)],
                                 ics[:, :], start=True, stop=True)
            nc.scalar.activation(
                dst[:, :].rearrange("p (g c) -> p g c", c=N_I),
                p8[:, :].rearrange("p (g c) -> p g c", c=512)[:, :, 0:N_I],
                AF.Copy)
        st["q"] = q
        st["k"] = k

    def ph_v():
        # v feature-major -> qv [128, (g, s, l-pad-32)] strided evac
        ics = st["ics"]
        v8 = ps.tile([128, 1024], f32, tag="mm8", bufs=1)
        for g in range(2):
            nc.tensor.matmul(v8[:, 512 * g:512 * g + N_I],
                             W[f"av_{nm}"][:, 128 * g:128 * (g + 1)],
                             ics[:, :], start=True, stop=True)
        dst = (qv[:, :].rearrange("p (g s m) -> p g s m", g=2, m=32)[:, :, :, 0:L])
        src = (v8[:, :].rearrange("p (g c) -> p g c", c=512)[:, :, 0:N_I]
               .rearrange("p g (s l) -> p g s l", l=L))
        nc.scalar.activation(dst, src, AF.Copy)

    def ph_trans():
        # 32x32 block transpose: vt[(h,m) at 32h, (g, s, d')]
        vt = sb.tile([128, 2 * 32 * S_I], bf16, tag="vt", bufs=4, name=f"vt{u}")
        nc.vector.transpose(vt[:, :], qv[:, :])
        st["vt"] = vt

    def ph_kbd():
        k = st["k"]
        for g in range(2):
            for h in range(4):
                kdst = (kbd[g][32 * h:32 * h + 32, :]
                        .rearrange("p (s m) -> p s m", m=128)[:, :, 32 * h:32 * h + L])
                ksrc = (k[32 * h:32 * h + 32, N_I * g:N_I * (g + 1)]
                        .rearrange("p (s m) -> p s m", m=L))
                nc.gpsimd.tensor_copy(kdst, ksrc)

    def ph_attn():
        q = st["q"]
        ebuf = sb.tile([128, 2 * N_I], bf16, tag="e", bufs=4, name=f"e{u}")
        rden = sb.tile([128, 2 * N_I], f32, tag="rden", bufs=4, name=f"rden{u}")
        sc8 = ps.tile([128, 1024], f32, tag="sc8", bufs=1)
        for g in range(2):
            for j in range(S_I):
                nc.tensor.matmul(
                    sc8[:, 512 * g + j * L:512 * g + (j + 1) * L],
                    kbd[g][:, 128 * j:128 * (j + 1)],
                    q[:, N_I * g + L * j:N_I * g + L * (j + 1)],
                    start=True, stop=True)
        nc.scalar.activation(
            ebuf[:, :].rearrange("p (g c) -> p g c", c=N_I),
            sc8[:, :].rearrange("p (g c) -> p g c", c=512)[:, :, 0:N_I],
            AF.Exp, scale=SCALE)
        for g in range(2):
            dn = ps.tile([128, N_I], f32, tag="dnav", bufs=2)
            nc.tensor.matmul(dn[:, :], W["ones_den"][:, :],
                             ebuf[:, N_I * g:N_I * (g + 1)], start=True, stop=True)
            nc.vector.reciprocal_approx_fast(rden[:, N_I * g:N_I * (g + 1)], dn[:, :])
        st["e"] = ebuf
        st["rden"] = rden

    def ph_av():
        vt, ebuf, rden = st["vt"], st["e"], st["rden"]
        avn = sb.tile([128, 2 * N_I], bf16, tag="avn", bufs=4, name=f"avn{u}")
        for g in range(2):
            av = ps.tile([128, N_I], f32, tag="dnav", bufs=2)
            for s in range(S_I):
                for h4 in range(4):
                    nc.tensor.matmul(
                        av[32 * h4:32 * h4 + 32, L * s:L * (s + 1)],
                        vt[32 * h4:32 * h4 + L,
                           512 * g + 32 * s:512 * g + 32 * s + 32],
                        ebuf[32 * h4:32 * h4 + L,
                             N_I * g + L * s:N_I * g + L * (s + 1)],
                        start=True, stop=True,
                        tile_position=(32 * h4, 32 * h4))
            nc.vector.tensor_mul(avn[:, N_I * g:N_I * (g + 1)], av[:, :],
                                 rden[:, N_I * g:N_I * (g + 1)])
        st["avn"] = avn

    def ph_proj(out_dma):
        avn = st["avn"]
        for t in range(2):
            pp = ps.tile([128, N_I], f32, tag="pp", bufs=1)
            nc.tensor.matmul(pp[:, :], W[f"proj_{nm}"][:, 128 * t:128 * (t + 1)],
                             avn[:, 0:N_I], start=True, stop=False)
            nc.tensor.matmul(pp[:, :], W[f"proj_{nm}"][:, 256 + 128 * t:256 + 128 * (t + 1)],
                             avn[:, N_I:2 * N_I], start=False, stop=False)
            nc.tensor.matmul(pp[:, :], W[f"wres_{nm}"][:, 128 * t:128 * (t + 1)],
                             ic[:, :], start=False, stop=True)
            o2 = sb.tile([128, N_I], bf16, tag=f"o2{t}", bufs=4, name=f"o2{t}_{u}")
            if t == 0:
                nc.scalar.activation(o2[:, :], pp[:, :], AF.Copy)
            else:
                nc.vector.tensor_copy(o2[:, :], pp[:, :])
            out_dma(t, o2[:, :])

    return [ph_yz, ph_var, ph_rsqrt, ph_scale, ph_qkv, ph_kbd, ph_v,
            ph_trans, ph_attn, ph_av, ph_proj]


def _build_kernel(nc, tc, b_loc, loop_tok=None, static_dma=False):
    import contextlib
    ctx = contextlib.ExitStack()
    n_tok = b_loc * L
    if loop_tok is None:
        loop_tok = n_tok

    dram = {}
    for nm in ("x", "y"):
        dram[f"ic_{nm}"] = nc.dram_tensor(f"ic_{nm}", [52, n_tok], bf16, kind="ExternalInput").ap()
        dram[f"mv_{nm}"] = nc.dram_tensor(f"mv_{nm}", [52, 52], bf16, kind="ExternalInput").ap()
        for key in ("aq", "ak", "av"):
            dram[f"{key}_{nm}"] = nc.dram_tensor(f"{key}_{nm}", [52, 256], bf16, kind="ExternalInput").ap()
        dram[f"wres_{nm}"] = nc.dram_tensor(f"wres_{nm}", [52, 256], bf16, kind="ExternalInput").ap()
        dram[f"wproj_{nm}"] = nc.dram_tensor(f"wproj_{nm}", [128, 512], bf16, kind="ExternalInput").ap()
    dram["ones_den"] = nc.dram_tensor("ones_den", [128, 128], bf16, kind="ExternalInput").ap()
    out_d = nc.dram_tensor("out", [2 * DIM, n_tok], bf16, kind="ExternalOutput").ap()

    const = ctx.enter_context(tc.tile_pool(name="const", bufs=1))
    sb = ctx.enter_context(tc.tile_pool(name="sb", bufs=1))
    ps = ctx.enter_context(tc.tile_pool(name="ps", bufs=2, space="PSUM"))

    W = {}
    for nm in ("x", "y"):
        for key, shp in (("mv", [52, 52]), ("aq", [52, 256]), ("ak", [52, 256]),
                         ("av", [52, 256]), ("wres", [52, 256]), ("proj", [128, 512])):
            dkey = f"wproj_{nm}" if key == "proj" else f"{key}_{nm}"
            W[f"{key}_{nm}"] = const.tile(shp, bf16, tag=f"{key}{nm}", name=f"{key}{nm}")
            nc.sync.dma_start(W[f"{key}_{nm}"][:, :], dram[dkey])
    W["ones_den"] = const.tile([128, 128], bf16, tag="ones_den", name="ones_den")
    nc.sync.dma_start(W["ones_den"][:, :], dram["ones_den"])
    W["ones52"] = const.tile([52, 52], bf16, tag="ones52", name="ones52")
    nc.vector.memset(W["ones52"][:, :], 1.0)
    W["magic"] = const.tile([52, N_I], u16, tag="magic", name="magic")
    nc.vector.memset(W["magic"][:, :], 0x5f37)
    # preamble dummy Exp: loads the exp_and_others act table so the loop
    # entry CFG-join knows it's resident (kills per-iteration table reloads)
    W["atl"] = const.tile([1, 1], bf16, tag="atl", name="atl")
    nc.vector.memset(W["atl"][:, :], 0.0)
    nc.scalar.activation(W["atl"][:, :], W["atl"][:, :], AF.Exp)

    # block-diag k staging + qv (v in l-pad-32 layout): preamble-zeroed;
    # loop bodies rewrite only the in-block columns, padding stays zero.
    # Shared between slices u and u+KQ_SHARE (dep tracking serializes safely).
    kbd, qvt = {}, {}
    for u in range(KQ_SHARE):
        for g in range(2):
            kbd[(g, u)] = const.tile([128, 128 * S_I], bf16, tag=f"kbd{g}{u}", name=f"kbd{g}{u}")
            nc.vector.memset(kbd[(g, u)][:, :], 0.0)
        qvt[u] = const.tile([128, 2 * 32 * S_I], bf16, tag=f"qv{u}", name=f"qv{u}")
        nc.vector.memset(qvt[u][:, :], 0.0)

    for nm in ("x", "y"):
        ob = 0 if nm == "x" else DIM
        with tc.For_i(0, loop_tok, TOK_B, name=f"chunks_{nm}", staggered_reset=True,
                      hint_engines=(mybir.EngineType.PE,)) as tok0:
            ics = []
            for uu in range(U):
                ict = sb.tile([52, N_I], bf16, tag=f"ic{uu}", bufs=2, name=f"ic{uu}")
                if static_dma:
                    nc.sync.dma_start(ict[:, :], dram[f"ic_{nm}"][:, uu * N_I:(uu + 1) * N_I])
                else:
                    nc.sync.dma_start(ict[:, :], dram[f"ic_{nm}"][:, ds(tok0 + uu * N_I, N_I)])
                ics.append(ict)

            def mk_out(uu):
                def out_dma(t, src_ap):
                    if static_dma:
                        nc.sync.dma_start(
                            out_d[ob + 128 * t: ob + 128 * (t + 1), uu * N_I:(uu + 1) * N_I],
                            src_ap)
                    else:
                        nc.sync.dma_start(
                            out_d[ob + 128 * t: ob + 128 * (t + 1), ds(tok0 + uu * N_I, N_I)],
                            src_ap)
                return out_dma

            phases = []
            for uu in range(U):
                us = uu % KQ_SHARE
                phs = _slice_phases(nc, sb, ps, W, nm, ics[uu], uu,
                                    [kbd[(0, us)], kbd[(1, us)]], qvt[us])
                od = mk_out(uu)
                phs[-1] = (lambda f=phs[-1], od=od: f(od))
                phases.append(phs)
            n_ph = len(phases[0])
            for slot in range(n_ph + STAGGER * (U - 1)):
                for uidx in range(U):
                    p = slot - STAGGER * uidx
                    if 0 <= p < n_ph:
                        phases[uidx][p]()
    ctx.close()


def _get_nc(b_loc, loop_tok=None, static_dma=False):
    key = (b_loc, loop_tok, static_dma)
    if key in _CACHE:
        return _CACHE[key]
    nc = bacc.Bacc("TRN2", target_bir_lowering=False, debug=False,
                   enable_asserts=False, num_devices=NCORES)
    with tile.TileContext(nc, trace_sim=False) as tc:
        _build_kernel(nc, tc, b_loc, loop_tok, static_dma)
    nc.compile()
    bass.Bass.finalize(nc)
    _CACHE[key] = nc
    return nc


def _in_maps(prep, b_loc, ncores):
    maps = []
    for c in range(ncores):
        s0 = c * b_loc
        m = {}
        for nm in ("x", "y"):
            ic = prep[f"ic_{nm}"][s0:s0 + b_loc].reshape(b_loc * L, 52).T
            m[f"ic_{nm}"] = _to_bf16(np.ascontiguousarray(ic))
            for key in ("mv", "aq", "ak", "av", "wres", "wproj"):
                m[f"{key}_{nm}"] = _to_bf16(prep[f"{key}_{nm}"])
        m["ones_den"] = _to_bf16(prep["ones_den"])
        maps.append(m)
    return maps


def kernel(**inputs):
    prep = _host_prep(inputs)
    nc = _get_nc(B_LOC)
    res = bass_utils.run_bass_kernel_spmd(nc, _in_maps(prep, B_LOC, NCORES),
                                          core_ids=list(range(NCORES)))
    outs = [res.results[c]["out"] for c in range(NCORES)]
    full = np.concatenate(
        [np.asarray(o, np.float32).reshape(2 * DIM, B_LOC, L).transpose(1, 0, 2)
         for o in outs], axis=0)
    return np.ascontiguousarray(full.reshape(B, 2 * DIM, 5, 5))


# revision 38
# speedup vs baseline: 1.0160x; 1.0160x over previous
"""Trainium2 Bass kernel for nn_Block_CD (dual-stream patch-embed + attention).

v13 design (per 16-sample slice, one stream; tokens t=(s,l), l=25):
  ic [52, 400]: im2col(3x3 conv, 27 rows) + one-hot position rows (25).
  var+eps = ic^T Mv ic (quadratic form; Mv folds mean, eps, 1/256)
    y = Mv @ ic (PE); z = ic*y (DVE); var = ones52 @ z (PE); evac (ACT)
  rs = rsqrt(var)/16 via bf16 fast-inverse-sqrt + 1 Newton step (DVE/Pool)
  ic_s = ic * rs  -> q,k,v matmuls contract 52 and come out LN-normalized
  q,k feature-major [128,(g,400)] from A_q,A_k (PE + plain evacs)
  kbd block-diag k built by strided bf16 copies (DVE 4x / Pool)
  scores: per-sample 128-contraction matmuls; E = exp(SCALE*sc) (ACT)
  den = ones_den @ E (PE); rden = 1/den (DVE); v token-major via ic32_s
  (32-padded) matmuls -> V4 [ (s4,m32), (j,f256) ]; av: per-(s,h) tiny
  matmuls lhsT=V4 slice, rhs=E slice; avn = av*rden (DVE)
  pp = proj^T avn + W_res^T ic (residual+proj_b folded); output DMA'd
  directly from PSUM as f32 (no o2 evac), host converts.
Sharding: pure data parallel, B=8192 over 8 cores.
"""
import sys
sys.path.insert(0, "/opt/trn_rl_repo")
import numpy as np
import ml_dtypes

import concourse.bass as bass
import concourse.mybir as mybir
import concourse.tile as tile
from concourse import bacc, bass_utils
from concourse.bass import ds

bf16 = mybir.dt.bfloat16
f32 = mybir.dt.float32
u16 = mybir.dt.uint16
AF = mybir.ActivationFunctionType
ALU = mybir.AluOpType

DIM = 256
HEADS = 8
HD = 32
L = 25
SCALE = HD ** -0.5
LN_EPS = 1e-5
NCORES = 8
B = 8192
B_LOC = B // NCORES

S_I = 16          # samples per inner slice
N_I = S_I * L     # 400 tokens
U = 32            # slices per hw-loop iteration
STAGGER = 2
KQ_SHARE = 2      # kbd/qv staging rings shared between slices u, u+KQ_SHARE
TOK_B = U * N_I   # 3200 tokens per For_i step

_CACHE = {}


def _to_bf16(a):
    return np.asarray(a, np.float32).astype(ml_dtypes.bfloat16)


def _host_prep(inputs):
    pos = np.asarray(inputs["pos_embed"], np.float64).reshape(L, DIM)
    ln_g = np.asarray(inputs["ln_g"], np.float64)
    ln_b = np.asarray(inputs["ln_b"], np.float64)

    def im2col_ext(img):
        p = np.pad(np.asarray(img, np.float32), ((0, 0), (0, 0), (1, 1), (1, 1)))
        Bn = img.shape[0]
        cols = np.empty((Bn, L, 52), np.float32)
        idx = 0
        for c in range(3):
            for di in range(3):
                for dj in range(3):
                    cols[:, :, idx] = p[:, c, di:di + 5, dj:dj + 5].reshape(Bn, L)
                    idx += 1
        cols[:, :, 27:] = np.eye(L, dtype=np.float32)[None]
        return cols  # [B, 25, 52]

    prep = {}
    for nm, ik, cw, cb, qw, pw, pb in (
        ("x", "x", "conv1_w", "conv1_b", "qkv_x_w", "proj_x_w", "proj_x_b"),
        ("y", "y", "conv2_w", "conv2_b", "qkv_y_w", "proj_y_w", "proj_y_b"),
    ):
        conv_w = np.asarray(inputs[cw], np.float64)
        conv_b = np.asarray(inputs[cb], np.float64)
        qkv_w = np.asarray(inputs[qw], np.float64)
        proj_w = np.asarray(inputs[pw], np.float64)
        proj_b = np.asarray(inputs[pb], np.float64)

        w_emb = np.empty((52, DIM), np.float64)
        w_emb[:27] = conv_w.reshape(DIM, 27).T
        w_emb[27:] = pos + conv_b[None, :]

        # quadratic-form variance matrix: var+eps = c^T Mv c
        m = w_emb.mean(axis=1)                     # [52] row means
        Mv = w_emb @ w_emb.T / DIM - np.outer(m, m)
        Mv[27:, 27:] += LN_EPS * np.eye(L)
        prep[f"mv_{nm}"] = Mv

        c = qkv_w @ ln_b
        assert np.abs(c).max() < 1e-6, "nonzero ln_b fold not supported"
        w_emb_c = w_emb - m[:, None]               # LN mean fold
        wqkv = w_emb_c @ (16.0 * qkv_w * ln_g[None, :]).T   # [52, 768]
        prep[f"aq_{nm}"] = wqkv[:, 0:256]
        prep[f"ak_{nm}"] = wqkv[:, 256:512]
        prep[f"av_{nm}"] = wqkv[:, 512:768]

        wp = proj_w.T                               # [256, 256] lhsT
        prep[f"wproj_{nm}"] = np.concatenate([wp[0:128], wp[128:256]], axis=1)

        w_res = np.empty((52, DIM), np.float64)     # residual + proj_b fold
        w_res[:27] = conv_w.reshape(DIM, 27).T
        w_res[27:] = (conv_b + proj_b)[None, :]
        prep[f"wres_{nm}"] = w_res

        prep[f"ic_{nm}"] = im2col_ext(inputs[ik])

    O = np.zeros((128, 128), np.float32)
    for h in range(4):
        O[h * HD:h * HD + L, h * HD:(h + 1) * HD] = 1.0
    prep["ones_den"] = O
    return prep


def _slice_phases(nc, sb, ps, W, nm, ic, u, kbd, qv):
    """Phase-emitter closures for one 16-sample slice of one stream."""
    st = {}

    def ph_var():
        # var+eps = ic^T Mv ic  (per token, replicated to 52 partitions)
        yq = ps.tile([52, N_I], f32, tag="st", bufs=1)
        nc.tensor.matmul(yq[:, :], W[f"mv_{nm}"][:, :], ic[:, :], start=True, stop=True)
        z = sb.tile([52, N_I], bf16, tag="z", bufs=4, name=f"z{u}")
        nc.vector.tensor_mul(z[:, :], ic[:, :], yq[:, :])
        vq = ps.tile([52, N_I], f32, tag="st", bufs=1)
        nc.tensor.matmul(vq[:, :], W["ones52"][:, :], z[:, :], start=True, stop=True)
        vb = sb.tile([52, N_I], bf16, tag="vb", bufs=4, name=f"vb{u}")
        nc.scalar.activation(vb[:, :], vq[:, :], AF.Copy)
        st["vb"] = vb

    def ph_rsqrt():
        vb = st["vb"]
        # bf16 fast inverse sqrt + 1 Newton step; rs = rsqrt(var+eps)/16
        y0 = sb.tile([52, N_I], bf16, tag="y0", bufs=4, name=f"y0{u}")
        t1 = sb.tile([52, N_I], bf16, tag="t1", bufs=4, name=f"t1{u}")
        rs = sb.tile([52, N_I], bf16, tag="rs", bufs=4, name=f"rs{u}")
        LSR = ALU.logical_shift_right
        nc.vector.tensor_scalar(y0[:, :].bitcast(u16), vb[:, :].bitcast(u16),
                                1, None, LSR)
        nc.gpsimd.tensor_sub(y0[:, :].bitcast(u16), W["magic"][:, :],
                             y0[:, :].bitcast(u16))
        nc.gpsimd.tensor_mul(t1[:, :], vb[:, :], y0[:, :])
        nc.gpsimd.tensor_mul(t1[:, :], t1[:, :], y0[:, :])
        nc.vector.tensor_scalar(t1[:, :], t1[:, :], -0.03125, 0.09375,
                                ALU.mult, ALU.add)
        nc.gpsimd.tensor_mul(rs[:, :], y0[:, :], t1[:, :])
        st["rs"] = rs

    def ph_scale():
        rs = st["rs"]
        ics = sb.tile([52, N_I], bf16, tag="ics", bufs=4, name=f"ics{u}")
        nc.gpsimd.tensor_mul(ics[:, :], ic[:, :], rs[:, :])
        st["ics"] = ics

    def ph_qkv():
        ics = st["ics"]
        q = sb.tile([128, 2 * N_I], bf16, tag="q", bufs=4, name=f"q{u}")
        k = sb.tile([128, 2 * N_I], bf16, tag="k", bufs=4, name=f"k{u}")
        for dst, wk in ((q, f"aq_{nm}"), (k, f"ak_{nm}")):
            p8 = ps.tile([128, 1024], f32, tag="mm8", bufs=1)
            for g in range(2):
                nc.tensor.matmul(p8[:, 512 * g:512 * g + N_I],
                                 W[wk][:, 128 * g:128 * (g + 1)],
                                 ics[:, :], start=True, stop=True)
            nc.scalar.activation(
                dst[:, :].rearrange("p (g c) -> p g c", c=N_I),
                p8[:, :].rearrange("p (g c) -> p g c", c=512)[:, :, 0:N_I],
                AF.Copy)
        st["q"] = q
        st["k"] = k

    def ph_v():
        # v feature-major -> qv [128, (g, s, l-pad-32)] strided evac
        ics = st["ics"]
        v8 = ps.tile([128, 1024], f32, tag="mm8", bufs=1)
        for g in range(2):
            nc.tensor.matmul(v8[:, 512 * g:512 * g + N_I],
                             W[f"av_{nm}"][:, 128 * g:128 * (g + 1)],
                             ics[:, :], start=True, stop=True)
        dst = (qv[:, :].rearrange("p (g s m) -> p g s m", g=2, m=32)[:, :, :, 0:L])
        src = (v8[:, :].rearrange("p (g c) -> p g c", c=512)[:, :, 0:N_I]
               .rearrange("p g (s l) -> p g s l", l=L))
        nc.scalar.activation(dst, src, AF.Copy)

    def ph_trans():
        # 32x32 block transpose: vt[(h,m) at 32h, (g, s, d')]
        vt = sb.tile([128, 2 * 32 * S_I], bf16, tag="vt", bufs=4, name=f"vt{u}")
        nc.vector.transpose(vt[:, :], qv[:, :])
        st["vt"] = vt

    def ph_kbd():
        k = st["k"]
        for g in range(2):
            for h in range(4):
                kdst = (kbd[g][32 * h:32 * h + 32, :]
                        .rearrange("p (s m) -> p s m", m=128)[:, :, 32 * h:32 * h + L])
                ksrc = (k[32 * h:32 * h + 32, N_I * g:N_I * (g + 1)]
                        .rearrange("p (s m) -> p s m", m=L))
                nc.gpsimd.tensor_copy(kdst, ksrc)

    def ph_attn():
        q = st["q"]
        ebuf = sb.tile([128, 2 * N_I], bf16, tag="e", bufs=4, name=f"e{u}")
        rden = sb.tile([128, 2 * N_I], f32, tag="rden", bufs=4, name=f"rden{u}")
        sc8 = ps.tile([128, 1024], f32, tag="sc8", bufs=1)
        for g in range(2):
            for j in range(S_I):
                nc.tensor.matmul(
                    sc8[:, 512 * g + j * L:512 * g + (j + 1) * L],
                    kbd[g][:, 128 * j:128 * (j + 1)],
                    q[:, N_I * g + L * j:N_I * g + L * (j + 1)],
                    start=True, stop=True)
        nc.scalar.activation(
            ebuf[:, :].rearrange("p (g c) -> p g c", c=N_I),
            sc8[:, :].rearrange("p (g c) -> p g c", c=512)[:, :, 0:N_I],
            AF.Exp, scale=SCALE)
        for g in range(2):
            dn = ps.tile([128, N_I], f32, tag="dnav", bufs=2)
            nc.tensor.matmul(dn[:, :], W["ones_den"][:, :],
                             ebuf[:, N_I * g:N_I * (g + 1)], start=True, stop=True)
            nc.vector.reciprocal_approx_fast(rden[:, N_I * g:N_I * (g + 1)], dn[:, :])
        st["e"] = ebuf
        st["rden"] = rden

    def ph_av():
        vt, ebuf, rden = st["vt"], st["e"], st["rden"]
        avn = sb.tile([128, 2 * N_I], bf16, tag="avn", bufs=4, name=f"avn{u}")
        for g in range(2):
            av = ps.tile([128, N_I], f32, tag="dnav", bufs=2)
            for s in range(S_I):
                for h4 in range(4):
                    nc.tensor.matmul(
                        av[32 * h4:32 * h4 + 32, L * s:L * (s + 1)],
                        vt[32 * h4:32 * h4 + L,
                           512 * g + 32 * s:512 * g + 32 * s + 32],
                        ebuf[32 * h4:32 * h4 + L,
                             N_I * g + L * s:N_I * g + L * (s + 1)],
                        start=True, stop=True,
                        tile_position=(32 * h4, 32 * h4))
            nc.vector.tensor_mul(avn[:, N_I * g:N_I * (g + 1)], av[:, :],
                                 rden[:, N_I * g:N_I * (g + 1)])
        st["avn"] = avn

    def ph_proj(out_dma):
        avn = st["avn"]
        for t in range(2):
            pp = ps.tile([128, N_I], f32, tag="pp", bufs=1)
            nc.tensor.matmul(pp[:, :], W[f"proj_{nm}"][:, 128 * t:128 * (t + 1)],
                             avn[:, 0:N_I], start=True, stop=False)
            nc.tensor.matmul(pp[:, :], W[f"proj_{nm}"][:, 256 + 128 * t:256 + 128 * (t + 1)],
                             avn[:, N_I:2 * N_I], start=False, stop=False)
            nc.tensor.matmul(pp[:, :], W[f"wres_{nm}"][:, 128 * t:128 * (t + 1)],
                             ic[:, :], start=False, stop=True)
            o2 = sb.tile([128, N_I], bf16, tag=f"o2{t}", bufs=4, name=f"o2{t}_{u}")
            if t == 0:
                nc.scalar.activation(o2[:, :], pp[:, :], AF.Copy)
            else:
                nc.vector.tensor_copy(o2[:, :], pp[:, :])
            out_dma(t, o2[:, :])

    return [ph_var, ph_rsqrt, ph_scale, ph_qkv, ph_v, ph_trans, ph_kbd,
            ph_attn, ph_av, ph_proj]


def _build_kernel(nc, tc, b_loc, loop_tok=None, static_dma=False):
    import contextlib
    ctx = contextlib.ExitStack()
    n_tok = b_loc * L
    if loop_tok is None:
        loop_tok = n_tok

    dram = {}
    for nm in ("x", "y"):
        dram[f"ic_{nm}"] = nc.dram_tensor(f"ic_{nm}", [52, n_tok], bf16, kind="ExternalInput").ap()
        dram[f"mv_{nm}"] = nc.dram_tensor(f"mv_{nm}", [52, 52], bf16, kind="ExternalInput").ap()
        for key in ("aq", "ak", "av"):
            dram[f"{key}_{nm}"] = nc.dram_tensor(f"{key}_{nm}", [52, 256], bf16, kind="ExternalInput").ap()
        dram[f"wres_{nm}"] = nc.dram_tensor(f"wres_{nm}", [52, 256], bf16, kind="ExternalInput").ap()
        dram[f"wproj_{nm}"] = nc.dram_tensor(f"wproj_{nm}", [128, 512], bf16, kind="ExternalInput").ap()
    dram["ones_den"] = nc.dram_tensor("ones_den", [128, 128], bf16, kind="ExternalInput").ap()
    out_d = nc.dram_tensor("out", [2 * DIM, n_tok], bf16, kind="ExternalOutput").ap()

    const = ctx.enter_context(tc.tile_pool(name="const", bufs=1))
    sb = ctx.enter_context(tc.tile_pool(name="sb", bufs=1))
    ps = ctx.enter_context(tc.tile_pool(name="ps", bufs=2, space="PSUM"))

    W = {}
    for nm in ("x", "y"):
        for key, shp in (("mv", [52, 52]), ("aq", [52, 256]), ("ak", [52, 256]),
                         ("av", [52, 256]), ("wres", [52, 256]), ("proj", [128, 512])):
            dkey = f"wproj_{nm}" if key == "proj" else f"{key}_{nm}"
            W[f"{key}_{nm}"] = const.tile(shp, bf16, tag=f"{key}{nm}", name=f"{key}{nm}")
            nc.sync.dma_start(W[f"{key}_{nm}"][:, :], dram[dkey])
    W["ones_den"] = const.tile([128, 128], bf16, tag="ones_den", name="ones_den")
    nc.sync.dma_start(W["ones_den"][:, :], dram["ones_den"])
    W["ones52"] = const.tile([52, 52], bf16, tag="ones52", name="ones52")
    nc.vector.memset(W["ones52"][:, :], 1.0)
    W["magic"] = const.tile([52, N_I], u16, tag="magic", name="magic")
    nc.vector.memset(W["magic"][:, :], 0x5f37)
    # preamble dummy Exp: loads the exp_and_others act table so the loop
    # entry CFG-join knows it's resident (kills per-iteration table reloads)
    W["atl"] = const.tile([1, 1], bf16, tag="atl", name="atl")
    nc.vector.memset(W["atl"][:, :], 0.0)
    nc.scalar.activation(W["atl"][:, :], W["atl"][:, :], AF.Exp)

    # block-diag k staging + qv (v in l-pad-32 layout): preamble-zeroed;
    # loop bodies rewrite only the in-block columns, padding stays zero.
    # Shared between slices u and u+KQ_SHARE (dep tracking serializes safely).
    kbd, qvt = {}, {}
    for u in range(KQ_SHARE):
        for g in range(2):
            kbd[(g, u)] = const.tile([128, 128 * S_I], bf16, tag=f"kbd{g}{u}", name=f"kbd{g}{u}")
            nc.vector.memset(kbd[(g, u)][:, :], 0.0)
        qvt[u] = const.tile([128, 2 * 32 * S_I], bf16, tag=f"qv{u}", name=f"qv{u}")
        nc.vector.memset(qvt[u][:, :], 0.0)

    for nm in ("x", "y"):
        ob = 0 if nm == "x" else DIM
        with tc.For_i(0, loop_tok, TOK_B, name=f"chunks_{nm}", staggered_reset=True,
                      hint_engines=(mybir.EngineType.PE,)) as tok0:
            ics = []
            for uu in range(U):
                ict = sb.tile([52, N_I], bf16, tag=f"ic{uu}", bufs=2, name=f"ic{uu}")
                if static_dma:
                    nc.sync.dma_start(ict[:, :], dram[f"ic_{nm}"][:, uu * N_I:(uu + 1) * N_I])
                else:
                    nc.sync.dma_start(ict[:, :], dram[f"ic_{nm}"][:, ds(tok0 + uu * N_I, N_I)])
                ics.append(ict)

            def mk_out(uu):
                def out_dma(t, src_ap):
                    if static_dma:
                        nc.sync.dma_start(
                            out_d[ob + 128 * t: ob + 128 * (t + 1), uu * N_I:(uu + 1) * N_I],
                            src_ap)
                    else:
                        nc.sync.dma_start(
                            out_d[ob + 128 * t: ob + 128 * (t + 1), ds(tok0 + uu * N_I, N_I)],
                            src_ap)
                return out_dma

            phases = []
            for uu in range(U):
                us = uu % KQ_SHARE
                phs = _slice_phases(nc, sb, ps, W, nm, ics[uu], uu,
                                    [kbd[(0, us)], kbd[(1, us)]], qvt[us])
                od = mk_out(uu)
                phs[-1] = (lambda f=phs[-1], od=od: f(od))
                phases.append(phs)
            n_ph = len(phases[0])
            for slot in range(n_ph + STAGGER * (U - 1)):
                for uidx in range(U):
                    p = slot - STAGGER * uidx
                    if 0 <= p < n_ph:
                        phases[uidx][p]()
    ctx.close()


def _get_nc(b_loc, loop_tok=None, static_dma=False):
    key = (b_loc, loop_tok, static_dma)
    if key in _CACHE:
        return _CACHE[key]
    nc = bacc.Bacc("TRN2", target_bir_lowering=False, debug=False,
                   enable_asserts=False, num_devices=NCORES)
    with tile.TileContext(nc, trace_sim=False) as tc:
        _build_kernel(nc, tc, b_loc, loop_tok, static_dma)
    nc.compile()
    bass.Bass.finalize(nc)
    _CACHE[key] = nc
    return nc


def _in_maps(prep, b_loc, ncores):
    maps = []
    for c in range(ncores):
        s0 = c * b_loc
        m = {}
        for nm in ("x", "y"):
            ic = prep[f"ic_{nm}"][s0:s0 + b_loc].reshape(b_loc * L, 52).T
            m[f"ic_{nm}"] = _to_bf16(np.ascontiguousarray(ic))
            for key in ("mv", "aq", "ak", "av", "wres", "wproj"):
                m[f"{key}_{nm}"] = _to_bf16(prep[f"{key}_{nm}"])
        m["ones_den"] = _to_bf16(prep["ones_den"])
        maps.append(m)
    return maps


def kernel(**inputs):
    prep = _host_prep(inputs)
    nc = _get_nc(B_LOC)
    res = bass_utils.run_bass_kernel_spmd(nc, _in_maps(prep, B_LOC, NCORES),
                                          core_ids=list(range(NCORES)))
    outs = [res.results[c]["out"] for c in range(NCORES)]
    full = np.concatenate(
        [np.asarray(o, np.float32).reshape(2 * DIM, B_LOC, L).transpose(1, 0, 2)
         for o in outs], axis=0)
    return np.ascontiguousarray(full.reshape(B, 2 * DIM, 5, 5))


# revision 44
# speedup vs baseline: 1.0499x; 1.0334x over previous
"""Trainium2 Bass kernel for nn_Block_CD (dual-stream patch-embed + attention).

v13 design (per 16-sample slice, one stream; tokens t=(s,l), l=25):
  ic [52, 400]: im2col(3x3 conv, 27 rows) + one-hot position rows (25).
  var+eps = ic^T Mv ic (quadratic form; Mv folds mean, eps, 1/256)
    y = Mv @ ic (PE); z = ic*y (DVE); var = ones52 @ z (PE); evac (ACT)
  rs = rsqrt(var)/16 via bf16 fast-inverse-sqrt + 1 Newton step (DVE/Pool)
  ic_s = ic * rs  -> q,k,v matmuls contract 52 and come out LN-normalized
  q,k feature-major [128,(g,400)] from A_q,A_k (PE + plain evacs)
  kbd block-diag k built by strided bf16 copies (DVE 4x / Pool)
  scores: per-sample 128-contraction matmuls; E = exp(SCALE*sc) (ACT)
  den = ones_den @ E (PE); rden = 1/den (DVE); v token-major via ic32_s
  (32-padded) matmuls -> V4 [ (s4,m32), (j,f256) ]; av: per-(s,h) tiny
  matmuls lhsT=V4 slice, rhs=E slice; avn = av*rden (DVE)
  pp = proj^T avn + W_res^T ic (residual+proj_b folded); output DMA'd
  directly from PSUM as f32 (no o2 evac), host converts.
Sharding: pure data parallel, B=8192 over 8 cores.
"""
import sys
sys.path.insert(0, "/opt/trn_rl_repo")
import numpy as np
import ml_dtypes

import concourse.bass as bass
import concourse.mybir as mybir
import concourse.tile as tile
from concourse import bacc, bass_utils
from concourse.bass import ds

bf16 = mybir.dt.bfloat16
f32 = mybir.dt.float32
u16 = mybir.dt.uint16
u32 = mybir.dt.uint32
AF = mybir.ActivationFunctionType
ALU = mybir.AluOpType

DIM = 256
HEADS = 8
HD = 32
L = 25
SCALE = HD ** -0.5
LN_EPS = 1e-5
NCORES = 8
B = 8192
B_LOC = B // NCORES

S_I = 16          # samples per inner slice
N_I = S_I * L     # 400 tokens
U = 32            # slices per hw-loop iteration
STAGGER = 2
KQ_SHARE = 2      # kbd/qv staging rings shared between slices u, u+KQ_SHARE
TOK_B = U * N_I   # 3200 tokens per For_i step

_CACHE = {}


def _to_bf16(a):
    return np.asarray(a, np.float32).astype(ml_dtypes.bfloat16)


def _host_prep(inputs):
    pos = np.asarray(inputs["pos_embed"], np.float64).reshape(L, DIM)
    ln_g = np.asarray(inputs["ln_g"], np.float64)
    ln_b = np.asarray(inputs["ln_b"], np.float64)

    def im2col_ext(img):
        p = np.pad(np.asarray(img, np.float32), ((0, 0), (0, 0), (1, 1), (1, 1)))
        Bn = img.shape[0]
        cols = np.empty((Bn, L, 52), np.float32)
        idx = 0
        for c in range(3):
            for di in range(3):
                for dj in range(3):
                    cols[:, :, idx] = p[:, c, di:di + 5, dj:dj + 5].reshape(Bn, L)
                    idx += 1
        cols[:, :, 27:] = np.eye(L, dtype=np.float32)[None]
        return cols  # [B, 25, 52]

    prep = {}
    for nm, ik, cw, cb, qw, pw, pb in (
        ("x", "x", "conv1_w", "conv1_b", "qkv_x_w", "proj_x_w", "proj_x_b"),
        ("y", "y", "conv2_w", "conv2_b", "qkv_y_w", "proj_y_w", "proj_y_b"),
    ):
        conv_w = np.asarray(inputs[cw], np.float64)
        conv_b = np.asarray(inputs[cb], np.float64)
        qkv_w = np.asarray(inputs[qw], np.float64)
        proj_w = np.asarray(inputs[pw], np.float64)
        proj_b = np.asarray(inputs[pb], np.float64)

        w_emb = np.empty((52, DIM), np.float64)
        w_emb[:27] = conv_w.reshape(DIM, 27).T
        w_emb[27:] = pos + conv_b[None, :]

        # quadratic-form variance matrix: var+eps = c^T Mv c
        # block-diag pair layout: two slices stacked at rows 0:52 / 64:116
        m = w_emb.mean(axis=1)                     # [52] row means
        Mv = w_emb @ w_emb.T / DIM - np.outer(m, m)
        Mv[27:, 27:] += LN_EPS * np.eye(L)
        Mv2 = np.zeros((116, 116))
        Mv2[0:52, 0:52] = Mv
        Mv2[64:116, 64:116] = Mv
        prep[f"mv_{nm}"] = Mv2

        c = qkv_w @ ln_b
        assert np.abs(c).max() < 1e-6, "nonzero ln_b fold not supported"
        w_emb_c = w_emb - m[:, None]               # LN mean fold
        wqkv = w_emb_c @ (16.0 * qkv_w * ln_g[None, :]).T   # [52, 768]

        def dup116(a):                             # rows 0:52 and 64:116
            d = np.zeros((116, a.shape[1]))
            d[0:52] = a
            d[64:116] = a
            return d
        prep[f"aq_{nm}"] = dup116(wqkv[:, 0:256])
        prep[f"ak_{nm}"] = dup116(wqkv[:, 256:512])
        prep[f"av_{nm}"] = dup116(wqkv[:, 512:768])

        wp = proj_w.T                               # [256, 256] lhsT
        prep[f"wproj_{nm}"] = np.concatenate([wp[0:128], wp[128:256]], axis=1)

        w_res = np.empty((52, DIM), np.float64)     # residual + proj_b fold
        w_res[:27] = conv_w.reshape(DIM, 27).T
        w_res[27:] = (conv_b + proj_b)[None, :]
        prep[f"wres_{nm}"] = dup116(w_res)

        prep[f"ic_{nm}"] = im2col_ext(inputs[ik])

    O = np.zeros((128, 128), np.float32)
    for h in range(4):
        O[h * HD:h * HD + L, h * HD:(h + 1) * HD] = 1.0
    prep["ones_den"] = O
    return prep


def _slice_phases(nc, sb, ps, W, nm, ic2, row0, u, kbd, qv, pst, lead):
    """Phase-emitter closures for one 16-sample slice of one stream.

    Stats (var/rsqrt/ic_s) are computed once per slice PAIR on a [116, N_I]
    partition-stacked layout (rows 0:52 = even slice, 64:116 = odd slice,
    block-diag Mv/ones): per-column engine cost is shared by both slices.
    `lead` emits the pair ops; the odd slice's stats phases are no-ops.
    """
    st = {}
    ic = ic2[row0:row0 + 52, :]

    def ph_var():
        if not lead:
            return
        yq = ps.tile([116, N_I], f32, tag="st", bufs=1)
        nc.tensor.matmul(yq[:, :], W[f"mv_{nm}"][:, :], ic2[:, :], start=True, stop=True)
        z = sb.tile([116, N_I], bf16, tag="z", bufs=4, name=f"z{u}")
        nc.vector.tensor_mul(z[:, :], ic2[:, :], yq[:, :])
        vq = ps.tile([116, N_I], f32, tag="st", bufs=1)
        nc.tensor.matmul(vq[:, :], W["ones52"][:, :], z[:, :], start=True, stop=True)
        vb = sb.tile([116, N_I], f32, tag="vb", bufs=4, name=f"vb{u}")
        nc.scalar.activation(vb[:, :], vq[:, :], AF.Copy)
        pst["vb"] = vb

    def ph_rsqrt():
        if not lead:
            return
        vb = pst["vb"]
        # f32 fast inverse sqrt + 1 Newton step; rs = rsqrt(var+eps)/16
        y0 = sb.tile([116, N_I], f32, tag="y0", bufs=4, name=f"y0{u}")
        t1 = sb.tile([116, N_I], f32, tag="t1", bufs=4, name=f"t1{u}")
        rs = sb.tile([116, N_I], f32, tag="rs", bufs=4, name=f"rs{u}")
        LSR = ALU.logical_shift_right
        nc.vector.tensor_scalar(y0[:, :].bitcast(u32), vb[:, :].bitcast(u32),
                                1, None, LSR)
        nc.gpsimd.tensor_sub(y0[:, :].bitcast(u32), W["magic"][:, :],
                             y0[:, :].bitcast(u32))
        nc.gpsimd.tensor_mul(t1[:, :], vb[:, :], y0[:, :])
        nc.gpsimd.tensor_mul(t1[:, :], t1[:, :], y0[:, :])
        nc.vector.tensor_scalar(t1[:, :], t1[:, :], -0.03125, 0.09375,
                                ALU.mult, ALU.add)
        nc.gpsimd.tensor_mul(rs[:, :], y0[:, :], t1[:, :])
        pst["rs"] = rs

    def ph_scale():
        if not lead:
            return
        rs = pst["rs"]
        ics2 = sb.tile([116, N_I], bf16, tag="ics", bufs=4, name=f"ics{u}")
        nc.gpsimd.tensor_mul(ics2[:, :], ic2[:, :], rs[:, :])
        pst["ics2"] = ics2

    def ph_qkv():
        ics = pst["ics2"][row0:row0 + 52, :]
        q = sb.tile([128, 2 * N_I], bf16, tag="q", bufs=4, name=f"q{u}")
        k = sb.tile([128, 2 * N_I], bf16, tag="k", bufs=4, name=f"k{u}")
        for dst, wk in ((q, f"aq_{nm}"), (k, f"ak_{nm}")):
            p8 = ps.tile([128, 1024], f32, tag="mm8", bufs=1)
            for g in range(2):
                nc.tensor.matmul(p8[:, 512 * g:512 * g + N_I],
                                 W[wk][row0:row0 + 52, 128 * g:128 * (g + 1)],
                                 ics[:, :], start=True, stop=True)
            nc.scalar.activation(
                dst[:, :].rearrange("p (g c) -> p g c", c=N_I),
                p8[:, :].rearrange("p (g c) -> p g c", c=512)[:, :, 0:N_I],
                AF.Copy)
        st["q"] = q
        st["k"] = k

    def ph_v():
        # v feature-major -> qv [128, (g, s, l-pad-32)] strided evac
        ics = pst["ics2"][row0:row0 + 52, :]
        v8 = ps.tile([128, 1024], f32, tag="mm8", bufs=1)
        for g in range(2):
            nc.tensor.matmul(v8[:, 512 * g:512 * g + N_I],
                             W[f"av_{nm}"][row0:row0 + 52, 128 * g:128 * (g + 1)],
                             ics[:, :], start=True, stop=True)
        dst = (qv[:, :].rearrange("p (g s m) -> p g s m", g=2, m=32)[:, :, :, 0:L])
        src = (v8[:, :].rearrange("p (g c) -> p g c", c=512)[:, :, 0:N_I]
               .rearrange("p g (s l) -> p g s l", l=L))
        nc.scalar.activation(dst, src, AF.Copy)

    def ph_trans():
        # 32x32 block transpose: vt[(h,m) at 32h, (g, s, d')]
        vt = sb.tile([128, 2 * 32 * S_I], bf16, tag="vt", bufs=4, name=f"vt{u}")
        nc.vector.transpose(vt[:, :], qv[:, :])
        st["vt"] = vt

    def ph_kbd():
        k = st["k"]
        for g in range(2):
            for h in range(4):
                kdst = (kbd[g][32 * h:32 * h + 32, :]
                        .rearrange("p (s m) -> p s m", m=128)[:, :, 32 * h:32 * h + L])
                ksrc = (k[32 * h:32 * h + 32, N_I * g:N_I * (g + 1)]
                        .rearrange("p (s m) -> p s m", m=L))
                nc.gpsimd.tensor_copy(kdst, ksrc)

    def ph_attn():
        q = st["q"]
        ebuf = sb.tile([128, 2 * N_I], bf16, tag="e", bufs=4, name=f"e{u}")
        rden = sb.tile([128, 2 * N_I], f32, tag="rden", bufs=4, name=f"rden{u}")
        sc8 = ps.tile([128, 1024], f32, tag="sc8", bufs=1)
        for g in range(2):
            for j in range(S_I):
                nc.tensor.matmul(
                    sc8[:, 512 * g + j * L:512 * g + (j + 1) * L],
                    kbd[g][:, 128 * j:128 * (j + 1)],
                    q[:, N_I * g + L * j:N_I * g + L * (j + 1)],
                    start=True, stop=True)
        nc.scalar.activation(
            ebuf[:, :].rearrange("p (g c) -> p g c", c=N_I),
            sc8[:, :].rearrange("p (g c) -> p g c", c=512)[:, :, 0:N_I],
            AF.Exp, scale=SCALE)
        for g in range(2):
            dn = ps.tile([128, N_I], f32, tag="dnav", bufs=2)
            nc.tensor.matmul(dn[:, :], W["ones_den"][:, :],
                             ebuf[:, N_I * g:N_I * (g + 1)], start=True, stop=True)
            nc.vector.reciprocal_approx_fast(rden[:, N_I * g:N_I * (g + 1)], dn[:, :])
        st["e"] = ebuf
        st["rden"] = rden

    def ph_av():
        vt, ebuf, rden = st["vt"], st["e"], st["rden"]
        avn = sb.tile([128, 2 * N_I], bf16, tag="avn", bufs=4, name=f"avn{u}")
        for g in range(2):
            av = ps.tile([128, N_I], f32, tag="dnav", bufs=2)
            for s in range(S_I):
                for h4 in range(4):
                    nc.tensor.matmul(
                        av[32 * h4:32 * h4 + 32, L * s:L * (s + 1)],
                        vt[32 * h4:32 * h4 + L,
                           512 * g + 32 * s:512 * g + 32 * s + 32],
                        ebuf[32 * h4:32 * h4 + L,
                             N_I * g + L * s:N_I * g + L * (s + 1)],
                        start=True, stop=True,
                        tile_position=(32 * h4, 32 * h4))
            nc.vector.tensor_mul(avn[:, N_I * g:N_I * (g + 1)], av[:, :],
                                 rden[:, N_I * g:N_I * (g + 1)])
        st["avn"] = avn

    def ph_proj(out_dma):
        avn = st["avn"]
        for t in range(2):
            pp = ps.tile([128, N_I], f32, tag="pp", bufs=1)
            nc.tensor.matmul(pp[:, :], W[f"proj_{nm}"][:, 128 * t:128 * (t + 1)],
                             avn[:, 0:N_I], start=True, stop=False)
            nc.tensor.matmul(pp[:, :], W[f"proj_{nm}"][:, 256 + 128 * t:256 + 128 * (t + 1)],
                             avn[:, N_I:2 * N_I], start=False, stop=False)
            nc.tensor.matmul(pp[:, :], W[f"wres_{nm}"][row0:row0 + 52, 128 * t:128 * (t + 1)],
                             ic[:, :], start=False, stop=True)
            o2 = sb.tile([128, N_I], bf16, tag=f"o2{t}", bufs=4, name=f"o2{t}_{u}")
            if t == 0:
                nc.scalar.activation(o2[:, :], pp[:, :], AF.Copy)
            else:
                nc.vector.tensor_copy(o2[:, :], pp[:, :])
            out_dma(t, o2[:, :])

    return [ph_var, ph_rsqrt, ph_scale, ph_qkv, ph_v, ph_trans, ph_kbd,
            ph_attn, ph_av, ph_proj]


def _build_kernel(nc, tc, b_loc, loop_tok=None, static_dma=False):
    import contextlib
    ctx = contextlib.ExitStack()
    n_tok = b_loc * L
    if loop_tok is None:
        loop_tok = n_tok

    dram = {}
    for nm in ("x", "y"):
        dram[f"ic_{nm}"] = nc.dram_tensor(f"ic_{nm}", [52, n_tok], bf16, kind="ExternalInput").ap()
        dram[f"mv_{nm}"] = nc.dram_tensor(f"mv_{nm}", [116, 116], bf16, kind="ExternalInput").ap()
        for key in ("aq", "ak", "av"):
            dram[f"{key}_{nm}"] = nc.dram_tensor(f"{key}_{nm}", [116, 256], bf16, kind="ExternalInput").ap()
        dram[f"wres_{nm}"] = nc.dram_tensor(f"wres_{nm}", [116, 256], bf16, kind="ExternalInput").ap()
        dram[f"wproj_{nm}"] = nc.dram_tensor(f"wproj_{nm}", [128, 512], bf16, kind="ExternalInput").ap()
    dram["ones_den"] = nc.dram_tensor("ones_den", [128, 128], bf16, kind="ExternalInput").ap()
    out_d = nc.dram_tensor("out", [2 * DIM, n_tok], bf16, kind="ExternalOutput").ap()

    const = ctx.enter_context(tc.tile_pool(name="const", bufs=1))
    sb = ctx.enter_context(tc.tile_pool(name="sb", bufs=1))
    ps = ctx.enter_context(tc.tile_pool(name="ps", bufs=2, space="PSUM"))

    W = {}
    for nm in ("x", "y"):
        for key, shp in (("mv", [116, 116]), ("aq", [116, 256]), ("ak", [116, 256]),
                         ("av", [116, 256]), ("wres", [116, 256]), ("proj", [128, 512])):
            dkey = f"wproj_{nm}" if key == "proj" else f"{key}_{nm}"
            W[f"{key}_{nm}"] = const.tile(shp, bf16, tag=f"{key}{nm}", name=f"{key}{nm}")
            nc.sync.dma_start(W[f"{key}_{nm}"][:, :], dram[dkey])
    W["ones_den"] = const.tile([128, 128], bf16, tag="ones_den", name="ones_den")
    nc.sync.dma_start(W["ones_den"][:, :], dram["ones_den"])
    W["ones52"] = const.tile([116, 116], bf16, tag="ones52", name="ones52")
    nc.vector.memset(W["ones52"][:, :], 0.0)
    nc.vector.memset(W["ones52"][0:52, 0:52], 1.0)
    nc.vector.memset(W["ones52"][64:116, 64:116], 1.0)
    W["magic"] = const.tile([116, N_I], u32, tag="magic", name="magic")
    nc.vector.memset(W["magic"][:, :], 0x5F3759DF)
    # preamble dummy Exp: loads the exp_and_others act table so the loop
    # entry CFG-join knows it's resident (kills per-iteration table reloads)
    W["atl"] = const.tile([1, 1], bf16, tag="atl", name="atl")
    nc.vector.memset(W["atl"][:, :], 0.0)
    nc.scalar.activation(W["atl"][:, :], W["atl"][:, :], AF.Exp)

    # block-diag k staging + qv (v in l-pad-32 layout): preamble-zeroed;
    # loop bodies rewrite only the in-block columns, padding stays zero.
    # Shared between slices u and u+KQ_SHARE (dep tracking serializes safely).
    kbd, qvt = {}, {}
    for u in range(KQ_SHARE):
        for g in range(2):
            kbd[(g, u)] = const.tile([128, 128 * S_I], bf16, tag=f"kbd{g}{u}", name=f"kbd{g}{u}")
            nc.vector.memset(kbd[(g, u)][:, :], 0.0)
        qvt[u] = const.tile([128, 2 * 32 * S_I], bf16, tag=f"qv{u}", name=f"qv{u}")
        nc.vector.memset(qvt[u][:, :], 0.0)

    for nm in ("x", "y"):
        ob = 0 if nm == "x" else DIM
        with tc.For_i(0, loop_tok, TOK_B, name=f"chunks_{nm}", staggered_reset=True,
                      hint_engines=(mybir.EngineType.PE,)) as tok0:
            ics = []
            for pu in range(U // 2):
                ict = sb.tile([116, N_I], bf16, tag=f"ic{pu}", bufs=2, name=f"ic{pu}")
                nc.gpsimd.memset(ict[32:64, :], 0.0)  # gap rows read by pair ops
                for half, uu in ((0, 2 * pu), (64, 2 * pu + 1)):
                    if static_dma:
                        nc.sync.dma_start(ict[half:half + 52, :],
                                          dram[f"ic_{nm}"][:, uu * N_I:(uu + 1) * N_I])
                    else:
                        nc.sync.dma_start(ict[half:half + 52, :],
                                          dram[f"ic_{nm}"][:, ds(tok0 + uu * N_I, N_I)])
                ics.append(ict)

            def mk_out(uu):
                def out_dma(t, src_ap):
                    if static_dma:
                        nc.sync.dma_start(
                            out_d[ob + 128 * t: ob + 128 * (t + 1), uu * N_I:(uu + 1) * N_I],
                            src_ap)
                    else:
                        nc.sync.dma_start(
                            out_d[ob + 128 * t: ob + 128 * (t + 1), ds(tok0 + uu * N_I, N_I)],
                            src_ap)
                return out_dma

            phases = []
            psts = [{} for _ in range(U // 2)]
            for uu in range(U):
                us = uu % KQ_SHARE
                phs = _slice_phases(nc, sb, ps, W, nm, ics[uu // 2],
                                    64 * (uu % 2), uu,
                                    [kbd[(0, us)], kbd[(1, us)]], qvt[us],
                                    psts[uu // 2], uu % 2 == 0)
                od = mk_out(uu)
                phs[-1] = (lambda f=phs[-1], od=od: f(od))
                phases.append(phs)
            n_ph = len(phases[0])
            for slot in range(n_ph + STAGGER * (U - 1)):
                for uidx in range(U):
                    p = slot - STAGGER * uidx
                    if 0 <= p < n_ph:
                        phases[uidx][p]()
    ctx.close()


def _get_nc(b_loc, loop_tok=None, static_dma=False):
    key = (b_loc, loop_tok, static_dma)
    if key in _CACHE:
        return _CACHE[key]
    nc = bacc.Bacc("TRN2", target_bir_lowering=False, debug=False,
                   enable_asserts=False, num_devices=NCORES)
    with tile.TileContext(nc, trace_sim=False) as tc:
        _build_kernel(nc, tc, b_loc, loop_tok, static_dma)
    nc.compile()
    bass.Bass.finalize(nc)
    _CACHE[key] = nc
    return nc


def _in_maps(prep, b_loc, ncores):
    maps = []
    for c in range(ncores):
        s0 = c * b_loc
        m = {}
        for nm in ("x", "y"):
            ic = prep[f"ic_{nm}"][s0:s0 + b_loc].reshape(b_loc * L, 52).T
            m[f"ic_{nm}"] = _to_bf16(np.ascontiguousarray(ic))
            for key in ("mv", "aq", "ak", "av", "wres", "wproj"):
                m[f"{key}_{nm}"] = _to_bf16(prep[f"{key}_{nm}"])
        m["ones_den"] = _to_bf16(prep["ones_den"])
        maps.append(m)
    return maps


def kernel(**inputs):
    prep = _host_prep(inputs)
    nc = _get_nc(B_LOC)
    res = bass_utils.run_bass_kernel_spmd(nc, _in_maps(prep, B_LOC, NCORES),
                                          core_ids=list(range(NCORES)))
    outs = [res.results[c]["out"] for c in range(NCORES)]
    full = np.concatenate(
        [np.asarray(o, np.float32).reshape(2 * DIM, B_LOC, L).transpose(1, 0, 2)
         for o in outs], axis=0)
    return np.ascontiguousarray(full.reshape(B, 2 * DIM, 5, 5))
